# revision 5
# baseline (speedup 1.0000x reference)
"""MoE FFN with hierarchical KV router — Trainium2 Bass kernel (8 NeuronCores).

v2 strategy (expert-parallel, weights loaded once per core):
  * Host computes the router in f64 (exact) and dispatches token slots by
    global expert id. All FFN FLOPs run on device.
  * Each core runs 3 "blocks" (SPMD — same program, different data):
      - 2 expert blocks in fp8-e4m3 with DoubleRow matmuls (2x PE rate).
        Core c owns experts 2c, 2c+1; each block processes CAPE token slots
        (CAPE = max expert load, rounded to 32). Expert quantization error
        (~3-4%) is scaled by sigmoid(gate_logit)≈0.119 in the combine, so the
        end-to-end contribution is ~0.4%.
      - 1 shared-FFN block in bf16: half of the H dimension (512) for 512
        tokens (core c: tokens chunk c//2, H-half c%2). The two halves of
        each token are summed on the host (which already does the combine).
  * All biases are folded out of the device program (b1 via the activation
    bias port, b2 added on host); fp8 scales travel in an aux tensor so the
    compiled program is scale-agnostic.
  * Outputs stream out in bf16; host does the final gather/combine in f32/f64.

Device layouts (per partition p, per core):
  eblob fp8 [2, 128, ECOLS]:
    XT  [0, 4*CAPE)        col k*CAPE+t   = Q8(sx * x[tok_t, k*128+p])
    W1  [XT_END, +4096)    col (m*2+kb)*256 + i*128 + q
                           = Q8(s1 * W1[kb*256 + i*128 + p, m*128+q])
    W2  [W1_END, +4096)    col (m2*4+kb2)*256 + i*128 + q
                           = Q8(s2 * W2[kb2*256 + i*128 + p, m2*128+q])
  eaux f32 [2, 128, 16]: cols 0..7 = SH*b1[m*128+p]; col 8 = SH/(sx*s1);
                         cols 9..15 = 0 (col 9 doubles as the zero scalar).
  sblob bf16 [128, 6144]:
    XS  [0, 2048)          col k*512+t = x[tok_t, k*128+p]
    W1S [2048, +2048)      col (m*4+k)*128+q  = sW1[k*128+p, hh*512 + m*128+q]
    W2S [4096, +2048)      col (m2*4+k2)*128+q = sW2[hh*512 + k2*128+p, m2*128+q]
  saux f32 [128, 8]: cols 0..3 = sb1[hh*512 + m*128+p]; rest 0.
  eout bf16 [2, 128, 4*CAPE]: col m2*CAPE+t = s_out * FFN_nobias(x)[t, m2*128+p]
  sout bf16 [128, 2048]:      col m2*512+t  = half-FFN(x)[t, m2*128+p]
"""
import sys

if "/opt/trn_rl_repo" not in sys.path:
    sys.path.insert(0, "/opt/trn_rl_repo")

import numpy as np

N_BUCKET, EPB, TOPK, TAU = 4, 4, 2, 1.0
C, H = 512, 1024
E = N_BUCKET * EPB
KC, KH, KHS = 4, 8, 4  # 128-blocks: C, H, H/2
N_CORES = 8
TOKS_S = 512  # shared tokens per core (half-H split)
SH = 16.0     # fp8 scale for h1

_BUILD_CACHE = {}


def _eoffsets(CAPE):
    XT = 0
    W1 = XT + KC * CAPE
    W2 = W1 + 4096
    ECOLS = W2 + 4096
    return XT, W1, W2, ECOLS


# sblob bf16 col offsets
SXS, SW1, SW2, SCOLS = 0, 2048, 4096, 6144


N_WARM = 11  # PE p-state warmup matmuls (~2.8us: bridges DMA-subsystem startup)


def _build_program(CAPE, has_bias=False):
    """Program order E0, S, E1. Expert blocks: fp8 DoubleRow FFN, CAPE tokens.
    Shared block: bf16 half-H FFN, 512 tokens.

    PSUM: 4 two-bank tensors A,B,C,D. mm1 m-block -> [A0,A512,B0,B512,C0,C512,
    A0,A512][m] (no intra-block reuse until m>=6, which waits only on act
    pair0). mm2 m2 -> [D0,D512,C0,C512][m2], kb2-outer so it consumes h1
    activation pairs as they land. Activations are double-width (one per
    psum-tensor pair) and alternate Scalar/DVE; psum->bf16 out copies split
    likewise; one output DMA per block on the Activation HWDGE queue.
    Input DMA issue (~0.6us each on the issuing sequencer) is spread:
    SP: aux+E0+S, GpSimd(SWDGE): E1, Activation: S pieces first, outs last."""
    from contextlib import ExitStack

    import concourse.bass as bass
    import concourse.mybir as mybir

    f32 = mybir.dt.float32
    bf16 = mybir.dt.bfloat16
    fp8 = mybir.dt.float8e4
    DR = mybir.MatmulPerfMode.DoubleRow
    XT, W1, W2, ECOLS = _eoffsets(CAPE)

    nc = bass.Bass("TRN2", target_bir_lowering=False, debug=False)
    eblob = nc.declare_dram_parameter("eblob", [2, 128, ECOLS], fp8, isOutput=False)
    aux = nc.declare_dram_parameter("aux", [128, 48], f32, isOutput=False)
    sblob = nc.declare_dram_parameter("sblob", [128, SCOLS], bf16, isOutput=False)
    eout = nc.declare_dram_parameter("eout", [2, 128, KC * CAPE], bf16, isOutput=True)
    sout = nc.declare_dram_parameter("sout", [128, KC * TOKS_S], bf16, isOutput=True)

    # program-order block descriptors: E0, S, E1
    blocks = [
        dict(kind="e", eb=0, KHb=KH, CAP=CAPE),
        dict(kind="s", eb=None, KHb=KHS, CAP=TOKS_S),
        dict(kind="e", eb=1, KHb=KH, CAP=CAPE),
    ]
    pe1base = [0, KH, KH + KHS]
    pe2base = [0, 4, 8]
    T1MAP = [0, 0, 1, 1, 2, 2, 0, 0]  # mm1 m -> psum tensor
    T2MAP = [3, 3, 2, 2]              # mm2 m2 -> psum tensor

    # Activation granularity: expert blocks use double-width acts over psum
    # pairs (group j = pair pp, covering m = 2pp, 2pp+1); the shared block
    # uses single-width acts (group j = m) so mm2's per-k2 data lands sooner.
    # Both alternate Scalar ("A") / DVE ("D"). has_bias: all acts
    # single-width on Scalar (only its activation has a bias port).
    def _singles(blk):
        return has_bias or blk["kind"] == "s"

    actidx = []  # [b][j] -> (semname, cumulative count)
    cntA = cntD = 0
    for b, blk in enumerate(blocks):
        row = []
        for j in range(blk["KHb"] if _singles(blk) else blk["KHb"] // 2):
            if has_bias or j % 2 == 0:
                cntA += 1
                row.append(("A", cntA))
            else:
                cntD += 1
                row.append(("D", cntD))
        actidx.append(row)

    def pair_done(b, pp):
        """Table entry implying acts for BOTH m = 2pp and 2pp+1 are done."""
        return actidx[b][2 * pp + 1] if _singles(blocks[b]) else actidx[b][pp]

    def h1_ready(b, kb2):
        """Entry for the h1 data consumed by mm2 contraction step kb2."""
        if blocks[b]["kind"] == "e":
            return pair_done(b, kb2)
        return actidx[b][kb2]  # shared: one h1 chunk per k2

    # mm1 bank-free waits: last reader of each psum (tensor, column-half)
    mm1wait = []  # [b][m] -> (semname, cnt) or None
    reader = {}
    for b, blk in enumerate(blocks):
        row = []
        for m in range(blk["KHb"]):
            if m >= 6:
                row.append(pair_done(b, 0))  # tensor A reused by pairs 0 and 3
            else:
                row.append(reader.get((T1MAP[m], m % 2)))
        mm1wait.append(row)
        for pp in range(blk["KHb"] // 2):
            t = T1MAP[2 * pp]
            reader[(t, 0)] = reader[(t, 1)] = pair_done(b, pp)
        # this block's mm2 also writes C (m2 2,3), read by its copy(-ies) —
        # the NEXT block's mm1 use of C must wait for those.
        if blk["kind"] == "e":
            reader[(2, 0)] = reader[(2, 1)] = ("cpA", cumA[b + 1])
        else:
            reader[(2, 0)] = ("cpA", cumA[b] + 1)
            reader[(2, 1)] = ("cpA", cumA[b] + 2)

    # mm2 bank-free waits per block: tensor D from prev cp0, C from prev cp1
    # (or, in expert blocks, C from this block's act pair2)
    # copy counts: expert blocks do one pair-copy per mm2 bank pair (cp0 -> D
    # on DVE, cp1 -> C on Scalar); the shared block does four single-width
    # copies (D halves on DVE, C halves consecutively on Scalar) so bank C
    # frees for E1's mm1 as soon as possible.
    cumD = [0]
    cumA = [0]
    for blk in blocks:
        n = 2 if blk["kind"] == "s" else 1
        cumD.append(cumD[-1] + n)
        cumA.append(cumA[-1] + n)

    mm2waitD = [None if b == 0 else ("cpD", cumD[b]) for b in range(3)]
    mm2waitC = []
    for b, blk in enumerate(blocks):
        if blk["kind"] == "e":
            mm2waitC.append(pair_done(b, 2))
        else:
            mm2waitC.append(None if b == 0 else ("cpA", cumA[b]))

    with ExitStack() as ctx:
        EB = [ctx.enter_context(nc.sbuf_tensor(f"eb{i}", [128, ECOLS], fp8)) for i in range(2)]
        AX = ctx.enter_context(nc.sbuf_tensor("ax", [128, 48], f32))
        SBt = ctx.enter_context(nc.sbuf_tensor("sb", [128, SCOLS], bf16))
        H1E = [ctx.enter_context(nc.sbuf_tensor(f"h1e{i}", [128, KH * CAPE], fp8)) for i in range(2)]
        H1S = ctx.enter_context(nc.sbuf_tensor("h1s", [128, KHS * TOKS_S], bf16))
        OTE = [ctx.enter_context(nc.sbuf_tensor(f"ote{i}", [128, KC * CAPE], bf16)) for i in range(2)]
        OTS = ctx.enter_context(nc.sbuf_tensor("ots", [128, KC * TOKS_S], bf16))
        JUNK = ctx.enter_context(nc.sbuf_tensor("junk", [128, 2 * CAPE], fp8))
        P4 = [ctx.enter_context(nc.psum_tensor(f"ps{i}", [128, 1024], f32)) for i in range(4)]
        auxS = ctx.enter_context(nc.semaphore("auxS"))
        in0 = [ctx.enter_context(nc.semaphore(f"in0_{b}")) for b in range(3)]
        in1 = [ctx.enter_context(nc.semaphore(f"in1_{b}")) for b in range(3)]
        in2 = [ctx.enter_context(nc.semaphore(f"in2_{b}")) for b in range(3)]
        in3 = [ctx.enter_context(nc.semaphore(f"in3_{b}")) for b in range(3)]
        pe1 = ctx.enter_context(nc.semaphore("pe1"))
        pe2 = ctx.enter_context(nc.semaphore("pe2"))
        actA = ctx.enter_context(nc.semaphore("actA"))
        actD = ctx.enter_context(nc.semaphore("actD"))
        cpA = ctx.enter_context(nc.semaphore("cpA"))
        cpD = ctx.enter_context(nc.semaphore("cpD"))
        outS = ctx.enter_context(nc.semaphore("outS"))
        jkS = ctx.enter_context(nc.semaphore("jkS"))
        block = ctx.enter_context(nc.Block(no_gpsimd_drain=True))

        SEMS = {"A": actA, "D": actD, "cpA": cpA, "cpD": cpD}

        def dr2(ap2d):
            """[128, 2*F] slice -> [128, 2, F] AP for DoubleRow."""
            return ap2d.rearrange("p (i f) -> p i f", i=2)

        def pspair(t, CAP):
            """Psum tensor t as [128, 2, CAP] (cols 0..CAP of each bank)."""
            ap = P4[t][:].rearrange("p (i c) -> p i c", i=2)
            return ap if CAP == 512 else ap[:, :, :CAP]

        class Waiter:
            """Emit monotone semaphore waits, skipping already-implied ones."""

            def __init__(self, eng):
                self.eng = eng
                self.seen = {}

            def w(self, sem, val):
                if val is None:
                    return
                if self.seen.get(id(sem), -1) < val:
                    self.eng.wait_ge(sem, val)
                    self.seen[id(sem)] = val

            def wp(self, pair):  # (semname, count) tuples from the tables
                if pair is not None:
                    self.w(SEMS[pair[0]], pair[1])

        @block.sync
        def _(sync):
            wt = Waiter(sync)
            bl = EB[0][:]
            sync.dma_start(out=bl[:, XT:W1 + 2048], in_=eblob[0][:, XT:W1 + 2048]).then_inc(in0[0], 16)
            sync.dma_start(out=bl[:, W1 + 2048:W2], in_=eblob[0][:, W1 + 2048:W2]).then_inc(in1[0], 16)
            sync.dma_start(out=bl[:, W2:W2 + 2048], in_=eblob[0][:, W2:W2 + 2048]).then_inc(in2[0], 16)
            sync.dma_start(out=bl[:, W2 + 2048:ECOLS], in_=eblob[0][:, W2 + 2048:ECOLS]).then_inc(in3[0], 16)
            sync.dma_start(out=AX[:], in_=aux[:, :]).then_inc(auxS, 16)
            # output DMAs: SP queue is idle once inputs are issued
            for b, blk in enumerate(blocks):
                wt.w(cpD, cumD[b + 1])
                if b < 2:
                    wt.w(cpA, cumA[b + 1])
                    if blk["kind"] == "e":
                        sync.dma_start(out=eout[blk["eb"]][:, :], in_=OTE[blk["eb"]][:]).then_inc(outS, 16)
                    else:
                        sync.dma_start(out=sout[:, :], in_=OTS[:]).then_inc(outS, 16)
                else:
                    # last block: first half issued as soon as cp0 is done so
                    # only the second half's issue+transfer is tail-exposed
                    e = blk["eb"]
                    half = 2 * blk["CAP"]
                    sync.dma_start(out=eout[e][:, 0:half], in_=OTE[e][:][:, 0:half]).then_inc(outS, 16)
                    wt.w(cpA, cumA[b + 1])
                    sync.dma_start(out=eout[e][:, half:2 * half], in_=OTE[e][:][:, half:2 * half]).then_inc(outS, 16)
            sync.wait_ge(outS, 16 * 4)

        @block.gpsimd
        def _(gpsimd):
            # stage E1's transfers behind S's first piece so they don't steal
            # DMA bandwidth from E0/S data that is needed sooner
            gpsimd.wait_ge(in0[1], 16)
            bl = EB[1][:]
            gpsimd.dma_start(out=bl[:, XT:W2], in_=eblob[1][:, XT:W2]).then_inc(in0[2], 16)
            gpsimd.dma_start(out=bl[:, W2:ECOLS], in_=eblob[1][:, W2:ECOLS]).then_inc(in2[2], 16)

        @block.tensor
        def _(tensor):
            wt = Waiter(tensor)
            tensor.wait_ge(jkS, 1)
            for _ in range(N_WARM):
                nc.tensor.matmul(
                    P4[0][:][:, :CAPE],
                    lhsT=dr2(JUNK[:][:, 0:256]),
                    rhs=dr2(JUNK[:][:, 0:2 * CAPE]),
                    start=True, stop=True, perf_mode=DR,
                )
            for b, blk in enumerate(blocks):
                kind, CAP, KHb = blk["kind"], blk["CAP"], blk["KHb"]
                K2 = KHS if kind == "s" else 4

                def emit_mm1(m, b=b, blk=blk):
                    kind, CAP = blk["kind"], blk["CAP"]
                    if b == 0 and m == 4:
                        wt.w(in1[b], 16)
                    wt.wp(mm1wait[b][m])
                    dst = P4[T1MAP[m]][:][:, (m % 2) * 512: (m % 2) * 512 + CAP]
                    if kind == "e":
                        bl = EB[blk["eb"]][:]
                        for kb in range(2):
                            mm = nc.tensor.matmul(
                                dst,
                                lhsT=dr2(bl[:, W1 + (m * 2 + kb) * 256: W1 + (m * 2 + kb) * 256 + 256]),
                                rhs=dr2(bl[:, XT + kb * 2 * CAP: XT + (kb * 2 + 2) * CAP]),
                                start=(kb == 0), stop=(kb == 1), perf_mode=DR,
                            )
                    else:
                        for k in range(KC):
                            mm = nc.tensor.matmul(
                                dst,
                                lhsT=SBt[:][:, SW1 + (m * 4 + k) * 128: SW1 + (m * 4 + k) * 128 + 128],
                                rhs=SBt[:][:, SXS + k * CAP: SXS + (k + 1) * CAP],
                                start=(k == 0), stop=(k == KC - 1),
                            )
                    mm.then_inc(pe1, 1)

                def emit_mm2(kb2, m2lo, m2hi, b=b, blk=blk, K2=K2):
                    kind, CAP = blk["kind"], blk["CAP"]
                    wt.wp(h1_ready(b, kb2))
                    for m2 in range(m2lo, m2hi):
                        if kb2 == 0:
                            wt.wp(mm2waitD[b] if m2 == 0 else None)
                            wt.wp(mm2waitC[b] if m2 == 2 else None)
                        dst = P4[T2MAP[m2]][:][:, (m2 % 2) * 512: (m2 % 2) * 512 + CAP]
                        if kind == "e":
                            bl, h1 = EB[blk["eb"]][:], H1E[blk["eb"]][:]
                            wcol = W2 + (m2 // 2) * 2048 + kb2 * 512 + (m2 % 2) * 256
                            mm = nc.tensor.matmul(
                                dst,
                                lhsT=dr2(bl[:, wcol: wcol + 256]),
                                rhs=dr2(h1[:, kb2 * 2 * CAP: (kb2 * 2 + 2) * CAP]),
                                start=(kb2 == 0), stop=(kb2 == K2 - 1), perf_mode=DR,
                            )
                        else:
                            wcol = SW2 + (m2 // 2) * 1024 + kb2 * 256 + (m2 % 2) * 128
                            mm = nc.tensor.matmul(
                                dst,
                                lhsT=SBt[:][:, wcol: wcol + 128],
                                rhs=H1S[:][:, kb2 * CAP: (kb2 + 1) * CAP],
                                start=(kb2 == 0), stop=(kb2 == K2 - 1),
                            )
                        if kb2 == K2 - 1:
                            mm.then_inc(pe2, 1)

                # mm2 runs in two kb2-outer sweeps: sweep 1 (m2 0,1 -> banks D)
                # consumes act pairs as they land, sweep 2 (m2 2,3 -> C) runs
                # with everything ready. For expert blocks, mm2's first
                # contraction step is interleaved before mm1's last psum-pair
                # (m6, m7) so the PE has work while act pair1 finishes.
                wt.w(in0[b], 16)
                if kind == "e":
                    for m in range(6):
                        emit_mm1(m)
                    wt.w(in2[b], 16)
                    emit_mm2(0, 0, 2)
                    emit_mm1(6)
                    emit_mm1(7)
                    for kb2 in range(1, K2):
                        emit_mm2(kb2, 0, 2)
                    if b == 0:
                        wt.w(in3[b], 16)
                    for kb2 in range(K2):
                        emit_mm2(kb2, 2, 4)
                else:
                    for m in range(KHb):
                        emit_mm1(m)
                    wt.w(in2[b], 16)
                    for m2lo, m2hi in ((0, 2), (2, 4)):
                        for kb2 in range(K2):
                            emit_mm2(kb2, m2lo, m2hi)

        def emit_act(eng, b, blk, pp):
            """Double-width relu over psum pair pp of an expert block."""
            CAP, e = blk["CAP"], blk["eb"]
            t = T1MAP[2 * pp]
            out = dr2(H1E[e][:][:, 2 * pp * CAP: (2 * pp + 2) * CAP])
            scale = AX[:][:, 16 * e + 8: 16 * e + 9]
            if eng is scalarE:
                return nc.scalar.activation(
                    out, pspair(t, CAP), mybir.ActivationFunctionType.Relu,
                    bias=AX[:][:, 9:10], scale=scale)
            return nc.vector.tensor_scalar(
                out, pspair(t, CAP), scale, 0.0,
                mybir.AluOpType.mult, mybir.AluOpType.max)

        def emit_act_s(eng, m):
            """Single-width shared-block relu for h1 chunk m."""
            t, col = T1MAP[m], (m % 2) * 512
            src = P4[t][:][:, col: col + 512]
            out = H1S[:][:, m * 512: (m + 1) * 512]
            if eng is scalarE:
                return nc.scalar.activation(
                    out, src, mybir.ActivationFunctionType.Relu,
                    bias=AX[:][:, 32 + m: 33 + m], scale=1.0)
            return nc.vector.tensor_scalar_max(out, src, 0.0)

        def emit_act_bias(b, blk, m):
            """Single-width relu with per-m bias on Scalar (has_bias path)."""
            kind, CAP = blk["kind"], blk["CAP"]
            t, col = T1MAP[m], (m % 2) * 512
            src = P4[t][:][:, col: col + CAP]
            if kind == "e":
                e = blk["eb"]
                return nc.scalar.activation(
                    H1E[e][:][:, m * CAP: (m + 1) * CAP], src,
                    mybir.ActivationFunctionType.Relu,
                    bias=AX[:][:, 16 * e + m: 16 * e + m + 1],
                    scale=AX[:][:, 16 * e + 8: 16 * e + 9])
            return nc.scalar.activation(
                H1S[:][:, m * CAP: (m + 1) * CAP], src,
                mybir.ActivationFunctionType.Relu,
                bias=AX[:][:, 32 + m: 33 + m], scale=1.0)

        def emit_copy(eng, b, blk, cp):
            """Psum->bf16 pair copy for an expert block (cp0 = D, cp1 = C)."""
            CAP = blk["CAP"]
            t = 3 if cp == 0 else 2
            dst = dr2(OTE[blk["eb"]][:][:, 2 * cp * CAP: (2 * cp + 2) * CAP])
            src = pspair(t, CAP)
            if eng is scalarE:
                return nc.scalar.copy(dst, src)
            return nc.vector.tensor_scalar_add(dst, src, AX[:][:, 9:10])

        def emit_copy_s(eng, m2):
            """Single-width psum->bf16 copy of shared mm2 output m2."""
            src = P4[T2MAP[m2]][:][:, (m2 % 2) * 512: (m2 % 2) * 512 + 512]
            dst = OTS[:][:, m2 * 512: (m2 + 1) * 512]
            if eng is scalarE:
                return nc.scalar.copy(dst, src)
            return nc.vector.tensor_scalar_add(dst, src, AX[:][:, 9:10])

        scalarE = object()  # sentinel token for engine selection
        vectorE = object()

        @block.scalar
        def _(scalar):
            wt = Waiter(scalar)
            # dummy act first: absorb the one-time Relu table load (~1.3us)
            # while E0's mm1 is still being fed. Reads memset JUNK; writes a
            # cell of OTE[0] that cp1 (same engine) later overwrites.
            wt.w(jkS, 1)
            nc.scalar.activation(
                OTE[0][:][:, 2 * blocks[0]["CAP"]: 2 * blocks[0]["CAP"] + 1],
                JUNK[:][:, 0:1], mybir.ActivationFunctionType.Relu,
                bias=JUNK[:][:, 0:4].bitcast(f32), scale=1.0)
            # stage S's transfers behind E0's w1 so E0's data lands first
            wt.w(in1[0], 16)
            scalar.dma_start(out=SBt[:][:, 0:SW2], in_=sblob[:, 0:SW2]).then_inc(in0[1], 16)
            scalar.dma_start(out=SBt[:][:, SW2:SCOLS], in_=sblob[:, SW2:SCOLS]).then_inc(in2[1], 16)
            wt.w(auxS, 16)
            for b, blk in enumerate(blocks):
                if has_bias:
                    for m in range(blk["KHb"]):
                        wt.w(pe1, pe1base[b] + m + 1)
                        emit_act_bias(b, blk, m).then_inc(actA, 1)
                elif blk["kind"] == "s":
                    for m in range(blk["KHb"]):
                        if m % 2 != 0:
                            continue
                        wt.w(pe1, pe1base[b] + m + 1)
                        emit_act_s(scalarE, m).then_inc(actA, 1)
                else:
                    for pp in range(blk["KHb"] // 2):
                        if pp % 2 != 0:
                            continue
                        wt.w(pe1, pe1base[b] + 2 * pp + 2)
                        emit_act(scalarE, b, blk, pp).then_inc(actA, 1)
                if blk["kind"] == "s":
                    wt.w(pe2, pe2base[b] + 3)
                    emit_copy_s(scalarE, 2).then_inc(cpA, 1)
                    wt.w(pe2, pe2base[b] + 4)
                    emit_copy_s(scalarE, 3).then_inc(cpA, 1)
                else:
                    wt.w(pe2, pe2base[b] + 4)
                    emit_copy(scalarE, b, blk, 1).then_inc(cpA, 1)


        @block.vector
        def _(vector):
            wt = Waiter(vector)
            nc.vector.memset(JUNK[:], 0).then_inc(jkS, 1)
            wt.w(auxS, 16)
            for b, blk in enumerate(blocks):
                if not has_bias and blk["kind"] == "s":
                    for m in range(blk["KHb"]):
                        if m % 2 != 1:
                            continue
                        wt.w(pe1, pe1base[b] + m + 1)
                        emit_act_s(vectorE, m).then_inc(actD, 1)
                elif not has_bias:
                    for pp in range(blk["KHb"] // 2):
                        if pp % 2 != 1:
                            continue
                        wt.w(pe1, pe1base[b] + 2 * pp + 2)
                        emit_act(vectorE, b, blk, pp).then_inc(actD, 1)
                if blk["kind"] == "s":
                    wt.w(pe2, pe2base[b] + 1)
                    emit_copy_s(vectorE, 0).then_inc(cpD, 1)
                    wt.w(pe2, pe2base[b] + 2)
                    emit_copy_s(vectorE, 1).then_inc(cpD, 1)
                else:
                    wt.w(pe2, pe2base[b] + 2)
                    emit_copy(vectorE, b, blk, 0).then_inc(cpD, 1)

    return nc


def _route(x2, bucket, expert_key):
    """Host router in float64. Returns gid (N,2), combine weights (N,2)."""
    hn = x2 / np.maximum(np.linalg.norm(x2, axis=-1, keepdims=True), 1e-12)
    keys = expert_key / np.maximum(
        np.linalg.norm(expert_key, axis=-1, keepdims=True), 1e-12
    )
    kb = keys[bucket]  # (N, EPB, C)
    score = np.einsum("nc,nec->ne", hn, kb) / max(TAU, 1e-6)
    score -= score.max(axis=-1, keepdims=True)
    p = np.exp(score)
    p /= p.sum(axis=-1, keepdims=True)
    local = np.argsort(-p, axis=-1, kind="stable")[:, :TOPK]  # (N, 2)
    topv = np.take_along_axis(p, local, axis=-1)
    w = topv / (topv.sum(axis=-1, keepdims=True) + 1e-9)
    gid = bucket[:, None] * EPB + local
    return gid, w


def _q8(a, ml):
    return np.ascontiguousarray(a).astype(ml.float8_e4m3)


def _fit_scale(a, nominal):
    amax = float(np.abs(a).max())
    if amax <= 0:
        return nominal
    return min(nominal, 224.0 / amax)


def _pack(inputs):
    """Returns (CAPE, per-core in_maps, unshard metadata)."""
    import ml_dtypes as ml

    x = np.asarray(inputs["x"], dtype=np.float32)
    op_id = np.asarray(inputs["op_id"]).astype(np.int64)
    expert_key = np.asarray(inputs["expert_key"], dtype=np.float64)
    sW1 = np.asarray(inputs["sW1"], dtype=np.float32)
    sb1 = np.asarray(inputs["sb1"], dtype=np.float32)
    sW2 = np.asarray(inputs["sW2"], dtype=np.float32)
    sb2 = np.asarray(inputs["sb2"], dtype=np.float32)
    eW1 = np.asarray(inputs["eW1"], dtype=np.float32)
    eb1 = np.asarray(inputs["eb1"], dtype=np.float32)
    eW2 = np.asarray(inputs["eW2"], dtype=np.float32)
    eb2 = np.asarray(inputs["eb2"], dtype=np.float32)
    gate_logit = float(np.asarray(inputs["gate_logit"]))

    B, T, Cc = x.shape
    assert Cc == C
    N = B * T
    assert N == (N_CORES // 2) * TOKS_S  # 2048 = 4 token chunks x 512
    x2 = x.reshape(N, C)
    bucket = np.clip(op_id.reshape(-1), 0, N_BUCKET - 1)

    gid, w = _route(x2.astype(np.float64), bucket, expert_key)
    gate = 1.0 / (1.0 + np.exp(-gate_logit))

    flat_gid = gid.reshape(-1)  # slot i -> token i//2
    sorted_slots = np.argsort(flat_gid, kind="stable")
    counts = np.bincount(flat_gid, minlength=E)
    CAPE = int(-(-counts.max() // 32) * 32)

    sx = _fit_scale(x2, 32.0)
    x8 = _q8(x2.T * sx, ml)  # (C, N) fp8
    xbf = x2.T.astype(ml.bfloat16)  # (C, N)

    XTo, W1o, W2o, ECOLS = _eoffsets(CAPE)

    in_maps = []
    s2s = np.zeros(E, np.float64)
    tok_of = np.zeros((E, CAPE), np.int64)  # token index per expert slot
    nslot = np.zeros(E, np.int64)
    slot_of = np.zeros((3, N), np.int64)  # (which expert-slot row for k=0,1)

    pos = 0
    exp_of = np.zeros((2, N), np.int64)
    for e in range(E):
        cnt = int(counts[e])
        slots_e = sorted_slots[pos: pos + cnt]
        pos += cnt
        toks = slots_e // TOPK
        tok_of[e, :cnt] = toks
        nslot[e] = cnt
        slot_of[slots_e % TOPK, toks] = e * CAPE + np.arange(cnt)
        exp_of[slots_e % TOPK, toks] = e

    has_bias = not (
        np.all(eb1 == 0.0) and np.all(sb1 == 0.0)
    )

    for c in range(N_CORES):
        eblob = np.zeros((2, 128, ECOLS), ml.float8_e4m3)
        aux = np.zeros((128, 48), np.float32)
        for b in range(2):
            e = 2 * c + b
            s1 = _fit_scale(eW1[e], 512.0)
            s2 = _fit_scale(eW2[e], 512.0)
            s2s[e] = s2
            cnt = int(nslot[e])
            toks = tok_of[e, :cnt]
            # XT
            xt = eblob[b, :, XTo:W1o].reshape(128, KC, CAPE)
            xt[:, :, :cnt] = x8[:, toks].reshape(KC, 128, cnt).transpose(1, 0, 2)
            # W1: [p, (m*2+kb)*256 + i*128 + q] = W1[kb*256+i*128+p, m*128+q]
            w1q = _q8(eW1[e] * s1, ml)
            eblob[b, :, W1o:W2o] = (
                w1q.reshape(2, 2, 128, KH, 128).transpose(2, 3, 0, 1, 4).reshape(128, 4096)
            )
            # W2 m2-pair-major: [p, (m2p*4+kb2)*512 + m2h*256 + i*128 + q]
            #   = W2[kb2*256+i*128+p, (m2p*2+m2h)*128+q]
            w2q = _q8(eW2[e] * s2, ml)
            eblob[b, :, W2o:ECOLS] = (
                w2q.reshape(4, 2, 128, 2, 2, 128).transpose(2, 3, 0, 4, 1, 5).reshape(128, 4096)
            )
            aux[:, 16 * b:16 * b + KH] = SH * eb1[e].reshape(KH, 128).T
            aux[:, 16 * b + 8] = SH / (sx * s1)

        hh, tc = c % 2, c // 2
        tokens = np.arange(tc * TOKS_S, (tc + 1) * TOKS_S)
        sblob = np.zeros((128, SCOLS), ml.bfloat16)
        sblob[:, SXS:SXS + KC * TOKS_S] = (
            xbf[:, tokens].reshape(KC, 128, TOKS_S).transpose(1, 0, 2).reshape(128, KC * TOKS_S)
        )
        w1h = sW1[:, hh * 512:(hh + 1) * 512].astype(ml.bfloat16)  # (C, 512)
        sblob[:, SW1:SW1 + 2048] = (
            w1h.reshape(KC, 128, KHS, 128).transpose(1, 2, 0, 3).reshape(128, 2048)
        )
        # W2S m2-pair-major: [p, (m2p*4+k2)*256 + m2h*128 + q]
        #   = sW2h[k2*128+p, (m2p*2+m2h)*128+q]
        w2h = sW2[hh * 512:(hh + 1) * 512, :].astype(ml.bfloat16)  # (512, C)
        sblob[:, SW2:SW2 + 2048] = (
            w2h.reshape(KHS, 128, 2, 2, 128).transpose(1, 2, 0, 3, 4).reshape(128, 2048)
        )
        aux[:, 32:32 + KHS] = sb1[hh * 512:(hh + 1) * 512].reshape(KHS, 128).T
        in_maps.append({"eblob": eblob, "aux": aux, "sblob": sblob})

    meta = dict(
        CAPE=CAPE, gate=gate, w=w, gid=gid, slot_of=slot_of, exp_of=exp_of,
        s2s=s2s, sx=sx, sb2=sb2, eb2=eb2, N=N, B=B, T=T, has_bias=has_bias,
    )
    return CAPE, in_maps, meta


def kernel(**inputs):
    import os

    from concourse.bass_utils import run_bass_kernel_spmd

    CAPE, in_maps, meta = _pack(inputs)

    key = (CAPE, meta["has_bias"])
    if key not in _BUILD_CACHE:
        _BUILD_CACHE[key] = _build_program(CAPE, has_bias=meta["has_bias"])
    nc = _BUILD_CACHE[key]

    trace = bool(os.environ.get("BASS_TRACE"))
    res = run_bass_kernel_spmd(
        nc,
        in_maps,
        core_ids=list(range(N_CORES)),
        trace=trace,
        trace_cores=list(range(N_CORES)) if trace else None,
    )
    global LAST_EXEC_NS, LAST_RESULTS
    LAST_EXEC_NS = res.exec_time_ns
    LAST_RESULTS = res

    return _combine(res.results, meta)


def _combine(results, meta):
    CAPEv = meta["CAPE"]
    N = meta["N"]
    # expert rows (scaled by SH*s2[e]): col m2*CAPE+t, C index m2*128+p
    allout = np.empty((E * CAPEv, C), np.float32)
    dense = np.zeros((N, C), np.float32)
    for c in range(N_CORES):
        eo = np.asarray(results[c]["eout"], dtype=np.float32).reshape(2, 128, KC, CAPEv)
        for b in range(2):
            e = 2 * c + b
            o = eo[b].transpose(2, 1, 0).reshape(CAPEv, C)  # (t, m2*128+p)
            allout[e * CAPEv: (e + 1) * CAPEv] = o / (SH * meta["s2s"][e])
        so = np.asarray(results[c]["sout"], dtype=np.float32).reshape(128, KC, TOKS_S)
        so = so.transpose(2, 1, 0).reshape(TOKS_S, C)  # (t, m2*128+p)
        tc = c // 2
        dense[tc * TOKS_S: (tc + 1) * TOKS_S] += so

    wf = (meta["gate"] * meta["w"]).astype(np.float32)  # (N, 2)
    slot_of = meta["slot_of"]
    exp_of = meta["exp_of"]
    y = (
        dense + meta["sb2"][None, :]
        + (allout[slot_of[0]] + meta["eb2"][exp_of[0]]) * wf[:, 0:1]
        + (allout[slot_of[1]] + meta["eb2"][exp_of[1]]) * wf[:, 1:2]
    )
    return y.reshape(meta["B"], meta["T"], C)


LAST_EXEC_NS = None
LAST_RESULTS = None


# revision 6
# speedup vs baseline: 1.0125x; 1.0125x over previous
"""MoE FFN with hierarchical KV router — Trainium2 Bass kernel (8 NeuronCores).

v2 strategy (expert-parallel, weights loaded once per core):
  * Host computes the router in f64 (exact) and dispatches token slots by
    global expert id. All FFN FLOPs run on device.
  * Each core runs 3 "blocks" (SPMD — same program, different data):
      - 2 expert blocks in fp8-e4m3 with DoubleRow matmuls (2x PE rate).
        Core c owns experts 2c, 2c+1; each block processes CAPE token slots
        (CAPE = max expert load, rounded to 32). Expert quantization error
        (~3-4%) is scaled by sigmoid(gate_logit)≈0.119 in the combine, so the
        end-to-end contribution is ~0.4%.
      - 1 shared-FFN block in bf16: half of the H dimension (512) for 512
        tokens (core c: tokens chunk c//2, H-half c%2). The two halves of
        each token are summed on the host (which already does the combine).
  * All biases are folded out of the device program (b1 via the activation
    bias port, b2 added on host); fp8 scales travel in an aux tensor so the
    compiled program is scale-agnostic.
  * Outputs stream out in bf16; host does the final gather/combine in f32/f64.

Device layouts (per partition p, per core):
  eblob fp8 [2, 128, ECOLS]:
    XT  [0, 4*CAPE)        col k*CAPE+t   = Q8(sx * x[tok_t, k*128+p])
    W1  [XT_END, +4096)    col (m*2+kb)*256 + i*128 + q
                           = Q8(s1 * W1[kb*256 + i*128 + p, m*128+q])
    W2  [W1_END, +4096)    col (m2*4+kb2)*256 + i*128 + q
                           = Q8(s2 * W2[kb2*256 + i*128 + p, m2*128+q])
  eaux f32 [2, 128, 16]: cols 0..7 = SH*b1[m*128+p]; col 8 = SH/(sx*s1);
                         cols 9..15 = 0 (col 9 doubles as the zero scalar).
  sblob bf16 [128, 6144]:
    XS  [0, 2048)          col k*512+t = x[tok_t, k*128+p]
    W1S [2048, +2048)      col (m*4+k)*128+q  = sW1[k*128+p, hh*512 + m*128+q]
    W2S [4096, +2048)      col (m2*4+k2)*128+q = sW2[hh*512 + k2*128+p, m2*128+q]
  saux f32 [128, 8]: cols 0..3 = sb1[hh*512 + m*128+p]; rest 0.
  eout bf16 [2, 128, 4*CAPE]: col m2*CAPE+t = s_out * FFN_nobias(x)[t, m2*128+p]
  sout bf16 [128, 2048]:      col m2*512+t  = half-FFN(x)[t, m2*128+p]
"""
import sys

if "/opt/trn_rl_repo" not in sys.path:
    sys.path.insert(0, "/opt/trn_rl_repo")

import numpy as np

N_BUCKET, EPB, TOPK, TAU = 4, 4, 2, 1.0
C, H = 512, 1024
E = N_BUCKET * EPB
KC, KH, KHS = 4, 8, 4  # 128-blocks: C, H, H/2
N_CORES = 8
TOKS_S = 512  # shared tokens per core (half-H split)
SH = 16.0     # fp8 scale for h1

_BUILD_CACHE = {}


def _eoffsets(CAPE):
    XT = 0
    W1 = XT + KC * CAPE
    W2 = W1 + 4096
    ECOLS = W2 + 4096
    return XT, W1, W2, ECOLS


# sblob bf16 col offsets
SXS, SW1, SW2, SCOLS = 0, 2048, 4096, 6144


N_WARM = 11  # PE p-state warmup matmuls (~2.8us: bridges DMA-subsystem startup)


def _build_program(CAPE, has_bias=False):
    """Program order E0, S, E1. Expert blocks: fp8 DoubleRow FFN, CAPE tokens.
    Shared block: bf16 half-H FFN, 512 tokens.

    PSUM: 4 two-bank tensors A,B,C,D. mm1 m-block -> [A0,A512,B0,B512,C0,C512,
    A0,A512][m] (no intra-block reuse until m>=6, which waits only on act
    pair0). mm2 m2 -> [D0,D512,C0,C512][m2], kb2-outer so it consumes h1
    activation pairs as they land. Activations are double-width (one per
    psum-tensor pair) and alternate Scalar/DVE; psum->bf16 out copies split
    likewise; one output DMA per block on the Activation HWDGE queue.
    Input DMA issue (~0.6us each on the issuing sequencer) is spread:
    SP: aux+E0+S, GpSimd(SWDGE): E1, Activation: S pieces first, outs last."""
    from contextlib import ExitStack

    import concourse.bass as bass
    import concourse.mybir as mybir

    f32 = mybir.dt.float32
    bf16 = mybir.dt.bfloat16
    fp8 = mybir.dt.float8e4
    DR = mybir.MatmulPerfMode.DoubleRow
    XT, W1, W2, ECOLS = _eoffsets(CAPE)

    nc = bass.Bass("TRN2", target_bir_lowering=False, debug=False)
    eblob = nc.declare_dram_parameter("eblob", [2, 128, ECOLS], fp8, isOutput=False)
    aux = nc.declare_dram_parameter("aux", [128, 48], f32, isOutput=False)
    sblob = nc.declare_dram_parameter("sblob", [128, SCOLS], bf16, isOutput=False)
    eout = nc.declare_dram_parameter("eout", [2, 128, KC * CAPE], bf16, isOutput=True)
    sout = nc.declare_dram_parameter("sout", [128, KC * TOKS_S], bf16, isOutput=True)

    # program-order block descriptors: E0, S, E1
    blocks = [
        dict(kind="e", eb=0, KHb=KH, CAP=CAPE),
        dict(kind="s", eb=None, KHb=KHS, CAP=TOKS_S),
        dict(kind="e", eb=1, KHb=KH, CAP=CAPE),
    ]
    pe1base = [0, KH, KH + KHS]
    pe2base = [0, 4, 8]
    T1MAP = [0, 0, 1, 1, 2, 2, 0, 0]  # mm1 m -> psum tensor
    T2MAP = [3, 3, 2, 2]              # mm2 m2 -> psum tensor

    # Activation granularity: expert blocks use double-width acts over psum
    # pairs (group j = pair pp, covering m = 2pp, 2pp+1); the shared block
    # uses single-width acts (group j = m) so mm2's per-k2 data lands sooner.
    # Both alternate Scalar ("A") / DVE ("D"). has_bias: all acts
    # single-width on Scalar (only its activation has a bias port).
    def _singles(blk):
        return has_bias or blk["kind"] == "s"

    actidx = []  # [b][j] -> (semname, cumulative count)
    cntA = cntD = 0
    for b, blk in enumerate(blocks):
        row = []
        for j in range(blk["KHb"] if _singles(blk) else blk["KHb"] // 2):
            if has_bias or j % 2 == 0:
                cntA += 1
                row.append(("A", cntA))
            else:
                cntD += 1
                row.append(("D", cntD))
        actidx.append(row)

    def pair_done(b, pp):
        """Table entry implying acts for BOTH m = 2pp and 2pp+1 are done."""
        return actidx[b][2 * pp + 1] if _singles(blocks[b]) else actidx[b][pp]

    def h1_ready(b, kb2):
        """Entry for the h1 data consumed by mm2 contraction step kb2."""
        if blocks[b]["kind"] == "e":
            return pair_done(b, kb2)
        return actidx[b][kb2]  # shared: one h1 chunk per k2

    # mm1 bank-free waits: last reader of each psum (tensor, column-half)
    mm1wait = []  # [b][m] -> (semname, cnt) or None
    reader = {}
    for b, blk in enumerate(blocks):
        row = []
        for m in range(blk["KHb"]):
            if m >= 6:
                row.append(pair_done(b, 0))  # tensor A reused by pairs 0 and 3
            else:
                row.append(reader.get((T1MAP[m], m % 2)))
        mm1wait.append(row)
        for pp in range(blk["KHb"] // 2):
            t = T1MAP[2 * pp]
            reader[(t, 0)] = reader[(t, 1)] = pair_done(b, pp)
        # this block's mm2 also writes C (m2 2,3), read by its copy(-ies) —
        # the NEXT block's mm1 use of C must wait for those.
        if blk["kind"] == "e":
            reader[(2, 0)] = reader[(2, 1)] = ("cpA", cumA[b + 1])
        else:
            reader[(2, 0)] = ("cpA", cumA[b] + 1)
            reader[(2, 1)] = ("cpA", cumA[b] + 2)

    # mm2 bank-free waits per block: tensor D from prev cp0, C from prev cp1
    # (or, in expert blocks, C from this block's act pair2)
    # copy counts: expert blocks do one pair-copy per mm2 bank pair (cp0 -> D
    # on DVE, cp1 -> C on Scalar); the shared block does four single-width
    # copies (D halves on DVE, C halves consecutively on Scalar) so bank C
    # frees for E1's mm1 as soon as possible.
    cumD = [0]
    cumA = [0]
    for blk in blocks:
        n = 2 if blk["kind"] == "s" else 1
        cumD.append(cumD[-1] + n)
        cumA.append(cumA[-1] + n)

    mm2waitD = [None if b == 0 else ("cpD", cumD[b]) for b in range(3)]
    mm2waitC = []
    for b, blk in enumerate(blocks):
        if blk["kind"] == "e":
            mm2waitC.append(pair_done(b, 2))
        else:
            mm2waitC.append(None if b == 0 else ("cpA", cumA[b]))

    with ExitStack() as ctx:
        EB = [ctx.enter_context(nc.sbuf_tensor(f"eb{i}", [128, ECOLS], fp8)) for i in range(2)]
        AX = ctx.enter_context(nc.sbuf_tensor("ax", [128, 48], f32))
        SBt = ctx.enter_context(nc.sbuf_tensor("sb", [128, SCOLS], bf16))
        H1E = [ctx.enter_context(nc.sbuf_tensor(f"h1e{i}", [128, KH * CAPE], fp8)) for i in range(2)]
        H1S = ctx.enter_context(nc.sbuf_tensor("h1s", [128, KHS * TOKS_S], bf16))
        OTE = [ctx.enter_context(nc.sbuf_tensor(f"ote{i}", [128, KC * CAPE], bf16)) for i in range(2)]
        OTS = ctx.enter_context(nc.sbuf_tensor("ots", [128, KC * TOKS_S], bf16))
        JUNK = ctx.enter_context(nc.sbuf_tensor("junk", [128, 2 * CAPE], fp8))
        P4 = [ctx.enter_context(nc.psum_tensor(f"ps{i}", [128, 1024], f32)) for i in range(4)]
        auxS = ctx.enter_context(nc.semaphore("auxS"))
        in0b = ctx.enter_context(nc.semaphore("in0b"))
        in0 = [ctx.enter_context(nc.semaphore(f"in0_{b}")) for b in range(3)]
        in1 = [ctx.enter_context(nc.semaphore(f"in1_{b}")) for b in range(3)]
        in2 = [ctx.enter_context(nc.semaphore(f"in2_{b}")) for b in range(3)]
        in3 = [ctx.enter_context(nc.semaphore(f"in3_{b}")) for b in range(3)]
        pe1 = ctx.enter_context(nc.semaphore("pe1"))
        pe2 = ctx.enter_context(nc.semaphore("pe2"))
        actA = ctx.enter_context(nc.semaphore("actA"))
        actD = ctx.enter_context(nc.semaphore("actD"))
        cpA = ctx.enter_context(nc.semaphore("cpA"))
        cpD = ctx.enter_context(nc.semaphore("cpD"))
        outS = ctx.enter_context(nc.semaphore("outS"))
        jkS = ctx.enter_context(nc.semaphore("jkS"))
        block = ctx.enter_context(nc.Block(no_gpsimd_drain=True))

        SEMS = {"A": actA, "D": actD, "cpA": cpA, "cpD": cpD}

        def dr2(ap2d):
            """[128, 2*F] slice -> [128, 2, F] AP for DoubleRow."""
            return ap2d.rearrange("p (i f) -> p i f", i=2)

        def pspair(t, CAP):
            """Psum tensor t as [128, 2, CAP] (cols 0..CAP of each bank)."""
            ap = P4[t][:].rearrange("p (i c) -> p i c", i=2)
            return ap if CAP == 512 else ap[:, :, :CAP]

        class Waiter:
            """Emit monotone semaphore waits, skipping already-implied ones."""

            def __init__(self, eng):
                self.eng = eng
                self.seen = {}

            def w(self, sem, val):
                if val is None:
                    return
                if self.seen.get(id(sem), -1) < val:
                    self.eng.wait_ge(sem, val)
                    self.seen[id(sem)] = val

            def wp(self, pair):  # (semname, count) tuples from the tables
                if pair is not None:
                    self.w(SEMS[pair[0]], pair[1])

        @block.sync
        def _(sync):
            wt = Waiter(sync)
            bl = EB[0][:]
            sync.dma_start(out=bl[:, XT:W1 + 2048], in_=eblob[0][:, XT:W1 + 2048]).then_inc(in0[0], 16)
            sync.dma_start(out=bl[:, W1 + 2048:W2], in_=eblob[0][:, W1 + 2048:W2]).then_inc(in1[0], 16)
            sync.dma_start(out=bl[:, W2:W2 + 2048], in_=eblob[0][:, W2:W2 + 2048]).then_inc(in2[0], 16)
            sync.dma_start(out=bl[:, W2 + 2048:ECOLS], in_=eblob[0][:, W2 + 2048:ECOLS]).then_inc(in3[0], 16)
            sync.dma_start(out=AX[:], in_=aux[:, :]).then_inc(auxS, 16)
            # output DMAs: SP queue is idle once inputs are issued
            for b, blk in enumerate(blocks):
                wt.w(cpD, cumD[b + 1])
                if b < 2:
                    wt.w(cpA, cumA[b + 1])
                    if blk["kind"] == "e":
                        sync.dma_start(out=eout[blk["eb"]][:, :], in_=OTE[blk["eb"]][:]).then_inc(outS, 16)
                    else:
                        sync.dma_start(out=sout[:, :], in_=OTS[:]).then_inc(outS, 16)
                else:
                    # last block: first half issued as soon as cp0 is done so
                    # only the second half's issue+transfer is tail-exposed
                    e = blk["eb"]
                    half = 2 * blk["CAP"]
                    sync.dma_start(out=eout[e][:, 0:half], in_=OTE[e][:][:, 0:half]).then_inc(outS, 16)
                    wt.w(cpA, cumA[b + 1])
                    sync.dma_start(out=eout[e][:, half:2 * half], in_=OTE[e][:][:, half:2 * half]).then_inc(outS, 16)
            sync.wait_ge(outS, 16 * 4)

        @block.gpsimd
        def _(gpsimd):
            # stage E1's transfers behind S's first piece so they don't steal
            # DMA bandwidth from E0/S data that is needed sooner
            gpsimd.wait_ge(in0[1], 16)
            bl = EB[1][:]
            gpsimd.dma_start(out=bl[:, XT:W2], in_=eblob[1][:, XT:W2]).then_inc(in0[2], 16)
            gpsimd.dma_start(out=bl[:, W2:ECOLS], in_=eblob[1][:, W2:ECOLS]).then_inc(in2[2], 16)

        @block.tensor
        def _(tensor):
            wt = Waiter(tensor)
            tensor.wait_ge(jkS, 1)
            for _ in range(N_WARM):
                nc.tensor.matmul(
                    P4[0][:][:, :CAPE],
                    lhsT=dr2(JUNK[:][:, 0:256]),
                    rhs=dr2(JUNK[:][:, 0:2 * CAPE]),
                    start=True, stop=True, perf_mode=DR,
                )
            for b, blk in enumerate(blocks):
                kind, CAP, KHb = blk["kind"], blk["CAP"], blk["KHb"]
                # ---- mm1 ----
                wt.w(in0[b], 16)
                for m in range(KHb):
                    if b == 0 and m == 4:
                        wt.w(in1[b], 16)
                    wt.wp(mm1wait[b][m])
                    dst = P4[T1MAP[m]][:][:, (m % 2) * 512: (m % 2) * 512 + CAP]
                    if kind == "e":
                        bl = EB[blk["eb"]][:]
                        for kb in range(2):
                            mm = nc.tensor.matmul(
                                dst,
                                lhsT=dr2(bl[:, W1 + (m * 2 + kb) * 256: W1 + (m * 2 + kb) * 256 + 256]),
                                rhs=dr2(bl[:, XT + kb * 2 * CAP: XT + (kb * 2 + 2) * CAP]),
                                start=(kb == 0), stop=(kb == 1), perf_mode=DR,
                            )
                    else:
                        for k in range(KC):
                            mm = nc.tensor.matmul(
                                dst,
                                lhsT=SBt[:][:, SW1 + (m * 4 + k) * 128: SW1 + (m * 4 + k) * 128 + 128],
                                rhs=SBt[:][:, SXS + k * CAP: SXS + (k + 1) * CAP],
                                start=(k == 0), stop=(k == KC - 1),
                            )
                    mm.then_inc(pe1, 1)
                # ---- mm2: two kb2-outer sweeps. Sweep 1 (m2 0,1 -> banks D)
                # needs only act pair kb2 as it goes; sweep 2 (m2 2,3 -> C)
                # then runs with all acts (and the C bank) long since ready,
                # so the PE never stalls mid-block. ----
                wt.w(in2[b], 16)
                K2 = KHS if kind == "s" else 4
                for m2lo, m2hi in ((0, 2), (2, 4)):
                    if b == 0 and m2lo == 2:
                        wt.w(in3[b], 16)
                    for kb2 in range(K2):
                        wt.wp(h1_ready(b, kb2))
                        for m2 in range(m2lo, m2hi):
                            if kb2 == 0:
                                wt.wp(mm2waitD[b] if m2 == 0 else None)
                                wt.wp(mm2waitC[b] if m2 == 2 else None)
                            dst = P4[T2MAP[m2]][:][:, (m2 % 2) * 512: (m2 % 2) * 512 + CAP]
                            if kind == "e":
                                bl, h1 = EB[blk["eb"]][:], H1E[blk["eb"]][:]
                                wcol = W2 + (m2 // 2) * 2048 + kb2 * 512 + (m2 % 2) * 256
                                mm = nc.tensor.matmul(
                                    dst,
                                    lhsT=dr2(bl[:, wcol: wcol + 256]),
                                    rhs=dr2(h1[:, kb2 * 2 * CAP: (kb2 * 2 + 2) * CAP]),
                                    start=(kb2 == 0), stop=(kb2 == K2 - 1), perf_mode=DR,
                                )
                            else:
                                wcol = SW2 + (m2 // 2) * 1024 + kb2 * 256 + (m2 % 2) * 128
                                mm = nc.tensor.matmul(
                                    dst,
                                    lhsT=SBt[:][:, wcol: wcol + 128],
                                    rhs=H1S[:][:, kb2 * CAP: (kb2 + 1) * CAP],
                                    start=(kb2 == 0), stop=(kb2 == K2 - 1),
                                )
                            if kb2 == K2 - 1:
                                mm.then_inc(pe2, 1)

        def emit_act(eng, b, blk, pp):
            """Double-width relu over psum pair pp of an expert block."""
            CAP, e = blk["CAP"], blk["eb"]
            t = T1MAP[2 * pp]
            out = dr2(H1E[e][:][:, 2 * pp * CAP: (2 * pp + 2) * CAP])
            scale = AX[:][:, 16 * e + 8: 16 * e + 9]
            if eng is scalarE:
                return nc.scalar.activation(
                    out, pspair(t, CAP), mybir.ActivationFunctionType.Relu,
                    bias=AX[:][:, 9:10], scale=scale)
            return nc.vector.tensor_scalar(
                out, pspair(t, CAP), scale, 0.0,
                mybir.AluOpType.mult, mybir.AluOpType.max)

        def emit_act_s(eng, m):
            """Single-width shared-block relu for h1 chunk m."""
            t, col = T1MAP[m], (m % 2) * 512
            src = P4[t][:][:, col: col + 512]
            out = H1S[:][:, m * 512: (m + 1) * 512]
            if eng is scalarE:
                return nc.scalar.activation(
                    out, src, mybir.ActivationFunctionType.Relu,
                    bias=AX[:][:, 32 + m: 33 + m], scale=1.0)
            return nc.vector.tensor_scalar_max(out, src, 0.0)

        def emit_act_bias(b, blk, m):
            """Single-width relu with per-m bias on Scalar (has_bias path)."""
            kind, CAP = blk["kind"], blk["CAP"]
            t, col = T1MAP[m], (m % 2) * 512
            src = P4[t][:][:, col: col + CAP]
            if kind == "e":
                e = blk["eb"]
                return nc.scalar.activation(
                    H1E[e][:][:, m * CAP: (m + 1) * CAP], src,
                    mybir.ActivationFunctionType.Relu,
                    bias=AX[:][:, 16 * e + m: 16 * e + m + 1],
                    scale=AX[:][:, 16 * e + 8: 16 * e + 9])
            return nc.scalar.activation(
                H1S[:][:, m * CAP: (m + 1) * CAP], src,
                mybir.ActivationFunctionType.Relu,
                bias=AX[:][:, 32 + m: 33 + m], scale=1.0)

        def emit_copy(eng, b, blk, cp):
            """Psum->bf16 pair copy for an expert block (cp0 = D, cp1 = C)."""
            CAP = blk["CAP"]
            t = 3 if cp == 0 else 2
            dst = dr2(OTE[blk["eb"]][:][:, 2 * cp * CAP: (2 * cp + 2) * CAP])
            src = pspair(t, CAP)
            if eng is scalarE:
                return nc.scalar.copy(dst, src)
            return nc.vector.tensor_scalar_add(dst, src, AX[:][:, 9:10])

        def emit_copy_s(eng, m2):
            """Single-width psum->bf16 copy of shared mm2 output m2."""
            src = P4[T2MAP[m2]][:][:, (m2 % 2) * 512: (m2 % 2) * 512 + 512]
            dst = OTS[:][:, m2 * 512: (m2 + 1) * 512]
            if eng is scalarE:
                return nc.scalar.copy(dst, src)
            return nc.vector.tensor_scalar_add(dst, src, AX[:][:, 9:10])

        scalarE = object()  # sentinel token for engine selection
        vectorE = object()

        @block.scalar
        def _(scalar):
            wt = Waiter(scalar)
            # dummy act first: absorb the one-time Relu table load (~1.3us)
            # while E0's mm1 is still being fed. Reads memset JUNK; writes a
            # cell of OTE[0] that cp1 (same engine) later overwrites.
            wt.w(jkS, 1)
            nc.scalar.activation(
                OTE[0][:][:, 2 * blocks[0]["CAP"]: 2 * blocks[0]["CAP"] + 1],
                JUNK[:][:, 0:1], mybir.ActivationFunctionType.Relu,
                bias=JUNK[:][:, 0:4].bitcast(f32), scale=1.0)
            # stage S's transfers behind E0's w1 so E0's data lands first
            wt.w(in1[0], 16)
            scalar.dma_start(out=SBt[:][:, 0:SW2], in_=sblob[:, 0:SW2]).then_inc(in0[1], 16)
            scalar.dma_start(out=SBt[:][:, SW2:SCOLS], in_=sblob[:, SW2:SCOLS]).then_inc(in2[1], 16)
            wt.w(auxS, 16)
            for b, blk in enumerate(blocks):
                if has_bias:
                    for m in range(blk["KHb"]):
                        wt.w(pe1, pe1base[b] + m + 1)
                        emit_act_bias(b, blk, m).then_inc(actA, 1)
                elif blk["kind"] == "s":
                    for m in range(blk["KHb"]):
                        if m % 2 != 0:
                            continue
                        wt.w(pe1, pe1base[b] + m + 1)
                        emit_act_s(scalarE, m).then_inc(actA, 1)
                else:
                    for pp in range(blk["KHb"] // 2):
                        if pp % 2 != 0:
                            continue
                        wt.w(pe1, pe1base[b] + 2 * pp + 2)
                        emit_act(scalarE, b, blk, pp).then_inc(actA, 1)
                if blk["kind"] == "s":
                    wt.w(pe2, pe2base[b] + 3)
                    emit_copy_s(scalarE, 2).then_inc(cpA, 1)
                    wt.w(pe2, pe2base[b] + 4)
                    emit_copy_s(scalarE, 3).then_inc(cpA, 1)
                else:
                    wt.w(pe2, pe2base[b] + 4)
                    emit_copy(scalarE, b, blk, 1).then_inc(cpA, 1)


        @block.vector
        def _(vector):
            wt = Waiter(vector)
            nc.vector.memset(JUNK[:], 0).then_inc(jkS, 1)
            wt.w(auxS, 16)
            for b, blk in enumerate(blocks):
                if not has_bias and blk["kind"] == "s":
                    for m in range(blk["KHb"]):
                        if m % 2 != 1:
                            continue
                        wt.w(pe1, pe1base[b] + m + 1)
                        emit_act_s(vectorE, m).then_inc(actD, 1)
                elif not has_bias:
                    for pp in range(blk["KHb"] // 2):
                        if pp % 2 != 1:
                            continue
                        wt.w(pe1, pe1base[b] + 2 * pp + 2)
                        emit_act(vectorE, b, blk, pp).then_inc(actD, 1)
                if blk["kind"] == "s":
                    wt.w(pe2, pe2base[b] + 1)
                    emit_copy_s(vectorE, 0).then_inc(cpD, 1)
                    wt.w(pe2, pe2base[b] + 2)
                    emit_copy_s(vectorE, 1).then_inc(cpD, 1)
                else:
                    wt.w(pe2, pe2base[b] + 2)
                    emit_copy(vectorE, b, blk, 0).then_inc(cpD, 1)

    return nc


def _route(x2, bucket, expert_key):
    """Host router in float64. Returns gid (N,2), combine weights (N,2)."""
    hn = x2 / np.maximum(np.linalg.norm(x2, axis=-1, keepdims=True), 1e-12)
    keys = expert_key / np.maximum(
        np.linalg.norm(expert_key, axis=-1, keepdims=True), 1e-12
    )
    kb = keys[bucket]  # (N, EPB, C)
    score = np.einsum("nc,nec->ne", hn, kb) / max(TAU, 1e-6)
    score -= score.max(axis=-1, keepdims=True)
    p = np.exp(score)
    p /= p.sum(axis=-1, keepdims=True)
    local = np.argsort(-p, axis=-1, kind="stable")[:, :TOPK]  # (N, 2)
    topv = np.take_along_axis(p, local, axis=-1)
    w = topv / (topv.sum(axis=-1, keepdims=True) + 1e-9)
    gid = bucket[:, None] * EPB + local
    return gid, w


def _q8(a, ml):
    return np.ascontiguousarray(a).astype(ml.float8_e4m3)


def _fit_scale(a, nominal):
    amax = float(np.abs(a).max())
    if amax <= 0:
        return nominal
    return min(nominal, 224.0 / amax)


def _pack(inputs):
    """Returns (CAPE, per-core in_maps, unshard metadata)."""
    import ml_dtypes as ml

    x = np.asarray(inputs["x"], dtype=np.float32)
    op_id = np.asarray(inputs["op_id"]).astype(np.int64)
    expert_key = np.asarray(inputs["expert_key"], dtype=np.float64)
    sW1 = np.asarray(inputs["sW1"], dtype=np.float32)
    sb1 = np.asarray(inputs["sb1"], dtype=np.float32)
    sW2 = np.asarray(inputs["sW2"], dtype=np.float32)
    sb2 = np.asarray(inputs["sb2"], dtype=np.float32)
    eW1 = np.asarray(inputs["eW1"], dtype=np.float32)
    eb1 = np.asarray(inputs["eb1"], dtype=np.float32)
    eW2 = np.asarray(inputs["eW2"], dtype=np.float32)
    eb2 = np.asarray(inputs["eb2"], dtype=np.float32)
    gate_logit = float(np.asarray(inputs["gate_logit"]))

    B, T, Cc = x.shape
    assert Cc == C
    N = B * T
    assert N == (N_CORES // 2) * TOKS_S  # 2048 = 4 token chunks x 512
    x2 = x.reshape(N, C)
    bucket = np.clip(op_id.reshape(-1), 0, N_BUCKET - 1)

    gid, w = _route(x2.astype(np.float64), bucket, expert_key)
    gate = 1.0 / (1.0 + np.exp(-gate_logit))

    flat_gid = gid.reshape(-1)  # slot i -> token i//2
    sorted_slots = np.argsort(flat_gid, kind="stable")
    counts = np.bincount(flat_gid, minlength=E)
    CAPE = int(-(-counts.max() // 32) * 32)

    sx = _fit_scale(x2, 32.0)
    x8 = _q8(x2.T * sx, ml)  # (C, N) fp8
    xbf = x2.T.astype(ml.bfloat16)  # (C, N)

    XTo, W1o, W2o, ECOLS = _eoffsets(CAPE)

    in_maps = []
    s2s = np.zeros(E, np.float64)
    tok_of = np.zeros((E, CAPE), np.int64)  # token index per expert slot
    nslot = np.zeros(E, np.int64)
    slot_of = np.zeros((3, N), np.int64)  # (which expert-slot row for k=0,1)

    pos = 0
    exp_of = np.zeros((2, N), np.int64)
    for e in range(E):
        cnt = int(counts[e])
        slots_e = sorted_slots[pos: pos + cnt]
        pos += cnt
        toks = slots_e // TOPK
        tok_of[e, :cnt] = toks
        nslot[e] = cnt
        slot_of[slots_e % TOPK, toks] = e * CAPE + np.arange(cnt)
        exp_of[slots_e % TOPK, toks] = e

    has_bias = not (
        np.all(eb1 == 0.0) and np.all(sb1 == 0.0)
    )

    for c in range(N_CORES):
        eblob = np.zeros((2, 128, ECOLS), ml.float8_e4m3)
        aux = np.zeros((128, 48), np.float32)
        for b in range(2):
            e = 2 * c + b
            s1 = _fit_scale(eW1[e], 512.0)
            s2 = _fit_scale(eW2[e], 512.0)
            s2s[e] = s2
            cnt = int(nslot[e])
            toks = tok_of[e, :cnt]
            # XT
            xt = eblob[b, :, XTo:W1o].reshape(128, KC, CAPE)
            xt[:, :, :cnt] = x8[:, toks].reshape(KC, 128, cnt).transpose(1, 0, 2)
            # W1: [p, (m*2+kb)*256 + i*128 + q] = W1[kb*256+i*128+p, m*128+q]
            w1q = _q8(eW1[e] * s1, ml)
            eblob[b, :, W1o:W2o] = (
                w1q.reshape(2, 2, 128, KH, 128).transpose(2, 3, 0, 1, 4).reshape(128, 4096)
            )
            # W2 m2-pair-major: [p, (m2p*4+kb2)*512 + m2h*256 + i*128 + q]
            #   = W2[kb2*256+i*128+p, (m2p*2+m2h)*128+q]
            w2q = _q8(eW2[e] * s2, ml)
            eblob[b, :, W2o:ECOLS] = (
                w2q.reshape(4, 2, 128, 2, 2, 128).transpose(2, 3, 0, 4, 1, 5).reshape(128, 4096)
            )
            aux[:, 16 * b:16 * b + KH] = SH * eb1[e].reshape(KH, 128).T
            aux[:, 16 * b + 8] = SH / (sx * s1)

        hh, tc = c % 2, c // 2
        tokens = np.arange(tc * TOKS_S, (tc + 1) * TOKS_S)
        sblob = np.zeros((128, SCOLS), ml.bfloat16)
        sblob[:, SXS:SXS + KC * TOKS_S] = (
            xbf[:, tokens].reshape(KC, 128, TOKS_S).transpose(1, 0, 2).reshape(128, KC * TOKS_S)
        )
        w1h = sW1[:, hh * 512:(hh + 1) * 512].astype(ml.bfloat16)  # (C, 512)
        sblob[:, SW1:SW1 + 2048] = (
            w1h.reshape(KC, 128, KHS, 128).transpose(1, 2, 0, 3).reshape(128, 2048)
        )
        # W2S m2-pair-major: [p, (m2p*4+k2)*256 + m2h*128 + q]
        #   = sW2h[k2*128+p, (m2p*2+m2h)*128+q]
        w2h = sW2[hh * 512:(hh + 1) * 512, :].astype(ml.bfloat16)  # (512, C)
        sblob[:, SW2:SW2 + 2048] = (
            w2h.reshape(KHS, 128, 2, 2, 128).transpose(1, 2, 0, 3, 4).reshape(128, 2048)
        )
        aux[:, 32:32 + KHS] = sb1[hh * 512:(hh + 1) * 512].reshape(KHS, 128).T
        in_maps.append({"eblob": eblob, "aux": aux, "sblob": sblob})

    meta = dict(
        CAPE=CAPE, gate=gate, w=w, gid=gid, slot_of=slot_of, exp_of=exp_of,
        s2s=s2s, sx=sx, sb2=sb2, eb2=eb2, N=N, B=B, T=T, has_bias=has_bias,
    )
    return CAPE, in_maps, meta


def kernel(**inputs):
    import os

    from concourse.bass_utils import run_bass_kernel_spmd

    CAPE, in_maps, meta = _pack(inputs)

    key = (CAPE, meta["has_bias"])
    if key not in _BUILD_CACHE:
        _BUILD_CACHE[key] = _build_program(CAPE, has_bias=meta["has_bias"])
    nc = _BUILD_CACHE[key]

    trace = bool(os.environ.get("BASS_TRACE"))
    res = run_bass_kernel_spmd(
        nc,
        in_maps,
        core_ids=list(range(N_CORES)),
        trace=trace,
        trace_cores=list(range(N_CORES)) if trace else None,
    )
    global LAST_EXEC_NS, LAST_RESULTS
    LAST_EXEC_NS = res.exec_time_ns
    LAST_RESULTS = res

    return _combine(res.results, meta)


def _combine(results, meta):
    CAPEv = meta["CAPE"]
    N = meta["N"]
    # expert rows (scaled by SH*s2[e]): col m2*CAPE+t, C index m2*128+p
    allout = np.empty((E * CAPEv, C), np.float32)
    dense = np.zeros((N, C), np.float32)
    for c in range(N_CORES):
        eo = np.asarray(results[c]["eout"], dtype=np.float32).reshape(2, 128, KC, CAPEv)
        for b in range(2):
            e = 2 * c + b
            o = eo[b].transpose(2, 1, 0).reshape(CAPEv, C)  # (t, m2*128+p)
            allout[e * CAPEv: (e + 1) * CAPEv] = o / (SH * meta["s2s"][e])
        so = np.asarray(results[c]["sout"], dtype=np.float32).reshape(128, KC, TOKS_S)
        so = so.transpose(2, 1, 0).reshape(TOKS_S, C)  # (t, m2*128+p)
        tc = c // 2
        dense[tc * TOKS_S: (tc + 1) * TOKS_S] += so

    wf = (meta["gate"] * meta["w"]).astype(np.float32)  # (N, 2)
    slot_of = meta["slot_of"]
    exp_of = meta["exp_of"]
    y = (
        dense + meta["sb2"][None, :]
        + (allout[slot_of[0]] + meta["eb2"][exp_of[0]]) * wf[:, 0:1]
        + (allout[slot_of[1]] + meta["eb2"][exp_of[1]]) * wf[:, 1:2]
    )
    return y.reshape(meta["B"], meta["T"], C)


LAST_EXEC_NS = None
LAST_RESULTS = None
